# revision 1
# baseline (speedup 1.0000x reference)
"""Criss-cross attention (CCAttention) Trainium2 kernel.

Shapes (hardcoded): x [8, 288, 128, 128] f32, Wq/Wk [36, 288], Wv [288, 288],
bq/bk [36], bv [288], eca_w [3], gamma [1]. Output [8, 288, 128, 128] f32.

Sharding: pure data parallel - one batch element per NeuronCore (8 cores).

Per-core algorithm (batch index dropped):
  q/k/v are 1x1 convs (channel GEMMs). Column attention couples pixels that
  share w; row attention couples pixels that share h; the two branches share
  a joint softmax over the concatenated 256 keys. Scores are small enough
  that exp() stays in fp32 range without max-subtraction, so each branch
  independently produces an unnormalized output U = sum exp(s) * v and a
  partition function Z = sum exp(s); the joint softmax is (UH+UW)/(ZH+ZW).
  Z rides as an extra column appended to the V tile in the AV matmul; that
  column holds 1/gamma instead of 1, so Z' = Z/gamma and the final scale
  gamma/Z is just reciprocal(Z').

  Phase 1 processes w in pairs: projections, column scores ST[h',h] =
  K_w.T Q_w, est = exp(ST) * (1-I), UH|ZH' = est.T @ [VT_w | 1/g]. The
  K=33 tail chunk of the 289-channel contraction for the two w of a pair
  runs concurrently in disjoint PE row-groups (partitions 0-63 / 64-127 via
  duplicated operands), as do the tail chunks of qk-projection subgroup
  pairs. The AV matmul lags the scores by one pair so the exp/mask latency
  hides under the V-transpose streams. VT and UH|ZH' stage to DRAM in bf16.

  Phase 2 loops over rows h: strided-row DMA reads of the staged tensors
  perform the spatial transpose; row scores use h-major copies of q/k so
  the stationary loads are contiguous; the row branch accumulates UW|ZW' on
  top of the loaded UH|ZH' via an identity matmul into the same PSUM, and
  the final combine og = (U * recip(Z')) + fac*x is done in [w, c] layout,
  with fac = 1 + gamma*sigmoid(eca(mean(x))) precomputed on the host.
  The host transposes [W,H,C] -> [C,H,W].

  Group sizes are graded ([8,8,16...] / [4,12,16...]) so the first DMAs are
  small and compute starts while the bulk loads stream in.
"""

import sys

sys.path.insert(0, "/opt/trn_rl_repo")

import numpy as np
import ml_dtypes

B, C, H, W = 8, 288, 128, 128
CQ = 36
KOFF = 64          # k block starts at column 64 of the packed qk weight
WVOFF = 100        # wv starts at column 100 of the merged weight tensor
N_PIX = H * W
BF16 = ml_dtypes.bfloat16

GW = 16  # max w-group size in phase 1
SW = 4   # qk projection subgroup (N = SW*128 = 512 per matmul)
GH = 16  # max h-group size in phase 2

P1_GROUPS = [8] + [16] * 7 + [8]
P2_GROUPS = [4, 12] + [16] * 7

KCH = [(0, 128), (128, 128), (256, C + 1 - 256)]
C3O, C3N = 256, C + 1 - 256   # tail chunk offset/size

_CACHE = {}


def _build_nc():
    import concourse.bass as bass
    import concourse.tile as tile
    import concourse.mybir as mybir
    from concourse import bacc
    from concourse.masks import make_identity

    f32 = mybir.dt.float32
    bf16 = mybir.dt.bfloat16
    AF = mybir.ActivationFunctionType

    nc = bacc.Bacc()

    # xw rows: 0:288 channels, 288 ones, 289:320 zero, 320:353 dup of 256:289
    # (the dup lands at tile partitions 64:97 so tail-chunk matmul pairs can
    # run in disjoint PE row-groups)
    xw = nc.dram_tensor("xw", [353, W, H], bf16, kind="ExternalInput")
    # xt is pre-multiplied by fac = 1 + gamma*sigmoid(eca(mean(x))) on host
    xt = nc.dram_tensor("xt", [W, H, C], bf16, kind="ExternalInput")
    # merged weights: cols 0:100 = packed qk (bias row at partition 288),
    # cols 100:388 = WvT; rows 320:353 dup rows 256:289
    wqv = nc.dram_tensor("wqv", [353, 388], bf16, kind="ExternalInput")
    rg = nc.dram_tensor("rg", [1, 1], f32, kind="ExternalInput")
    out = nc.dram_tensor("out", [W, H, C], bf16, kind="ExternalOutput")

    with tile.TileContext(nc) as tc:
        with tc.tile_pool(name="persist", bufs=1) as persist, \
             tc.tile_pool(name="dram", bufs=1, space="DRAM") as dpool:
            vt_st = dpool.tile([H, W, C + 1], bf16)   # [h', w, c | 1/gamma]
            uh_st = dpool.tile([H, W, C + 1], bf16)   # [h(query), w, c | ZH']

            q_sb = persist.tile([CQ, N_PIX], bf16)    # w-major: n = w*128 + h
            k_sb = persist.tile([CQ, N_PIX], bf16)
            ident = persist.tile([128, 128], f32)
            make_identity(nc, ident)
            identb = persist.tile([128, 128], bf16)
            nc.vector.tensor_copy(out=identb[:, :], in_=ident[:, :])
            # (1 - I) diagonal mask for the column-branch scores
            mask_sb = persist.tile([H, H], bf16)
            nc.scalar.activation(
                out=mask_sb[:, :], in_=ident[:, :], func=AF.Copy,
                scale=-1.0, bias=1.0,
            )

            wqv_sb = []
            for (ofs, cnt) in KCH[:2]:
                t = persist.tile([cnt, 388], bf16, tag=f"wqv{ofs}")
                nc.scalar.dma_start(out=t[:, :], in_=wqv[ofs:ofs + cnt, :])
                wqv_sb.append(t)
            # tail chunk with host-built dup at partitions 64:97
            wqv3d = persist.tile([64 + C3N, 388], bf16)
            nc.scalar.dma_start(out=wqv3d[:, :], in_=wqv[C3O:C3O + 97, :])
            wqv_sb.append(wqv3d)

            rgcol = persist.tile([128, GW], bf16)
            nc.gpsimd.dma_start(
                out=rgcol[:, :].rearrange("p (w o) -> p w o", o=1),
                in_=rg[0:1, 0:1].to_broadcast([128, GW, 1]),
            )

            q_v = q_sb[:, :].rearrange("p (w h) -> p h w", h=H)
            k_v = k_sb[:, :].rearrange("p (w h) -> p h w", h=H)

            # ---------------- Phase 1: column branch (per w) ----------------
            with tc.tile_pool(name="p1x", bufs=2) as p1x, \
                 tc.tile_pool(name="p1g", bufs=2) as p1g, \
                 tc.tile_pool(name="p1s", bufs=6) as p1s, \
                 tc.tile_pool(name="qkp", bufs=2, space="PSUM") as qkp, \
                 tc.tile_pool(name="vtp", bufs=2, space="PSUM") as vtp, \
                 tc.tile_pool(name="stp", bufs=2, space="PSUM") as stp, \
                 tc.tile_pool(name="uhp", bufs=2, space="PSUM") as uhp:
                w0 = 0
                for gw in P1_GROUPS:
                    xw_c = []
                    for j, (ofs, cnt) in enumerate(KCH[:2]):
                        t = p1x.tile([cnt, GW, H], bf16, tag=f"xw{j}")
                        eng = nc.sync if j == 0 else nc.scalar
                        eng.dma_start(
                            out=t[:, 0:gw, :], in_=xw[ofs:ofs + cnt, w0:w0 + gw, :]
                        )
                        xw_c.append(t)
                    # tail chunk with host-built dup at partitions 64:97
                    x3d = p1x.tile([64 + C3N, GW, H], bf16, tag="xw2")
                    nc.gpsimd.dma_start(
                        out=x3d[:, 0:gw, :], in_=xw[C3O:C3O + 97, w0:w0 + gw, :]
                    )

                    # q/k projections: subgroup pairs, tail chunks run as a
                    # concurrent row-group pair
                    ns = gw // SW
                    for sp in range(ns // 2):
                        s0, s1 = 2 * sp, 2 * sp + 1
                        ps = []
                        for s in (s0, s1):
                            qk_ps = qkp.tile([100, SW * H], f32, tag="qkp")
                            for j in range(2):
                                nc.tensor.matmul(
                                    qk_ps[:, :], wqv_sb[j][:, 0:100],
                                    xw_c[j][:, s * SW:(s + 1) * SW, :],
                                    start=(j == 0), stop=False,
                                )
                            ps.append(qk_ps)
                        nc.tensor.matmul(
                            ps[0][:, :], wqv3d[0:C3N, 0:100],
                            x3d[0:C3N, s0 * SW:(s0 + 1) * SW, :],
                            start=False, stop=True,
                        )
                        nc.tensor.matmul(
                            ps[1][:, :], wqv3d[64:64 + C3N, 0:100],
                            x3d[64:64 + C3N, s1 * SW:(s1 + 1) * SW, :],
                            start=False, stop=True,
                        )
                        for s, qk_ps in ((s0, ps[0]), (s1, ps[1])):
                            fo = (w0 + s * SW) * H
                            nc.vector.tensor_copy(
                                out=q_sb[:, fo:fo + SW * H], in_=qk_ps[0:CQ, :]
                            )
                            nc.scalar.copy(
                                out=k_sb[:, fo:fo + SW * H],
                                in_=qk_ps[KOFF:KOFF + CQ, :],
                            )

                    vtg = p1g.tile([128, GW, C + 1], bf16, tag="vtg")
                    uhg = p1g.tile([128, GW, C + 1], bf16, tag="uhg")
                    nc.vector.tensor_copy(
                        out=vtg[:, 0:gw, C:C + 1],
                        in_=rgcol[:, 0:gw].rearrange("p (w o) -> p w o", o=1),
                    )

                    # per-pair pipelined loop: VT + ST lead, AV lags one pair
                    ests = [None] * GW
                    npair = gw // 2
                    for pp in range(npair + 1):
                        if pp < npair:
                            we, wo = 2 * pp, 2 * pp + 1
                            # VT chunks; the two K=33 tails run concurrently
                            # in row-groups 0-63 / 64-127
                            vt_e = vtp.tile([128, C], f32, tag="vt")
                            vt_o = vtp.tile([128, C], f32, tag="vt")
                            for wi, vt_ps in ((we, vt_e), (wo, vt_o)):
                                for j in range(2):
                                    nc.tensor.matmul(
                                        vt_ps[:, :], xw_c[j][:, wi, :],
                                        wqv_sb[j][:, WVOFF:WVOFF + C],
                                        start=(j == 0), stop=False,
                                    )
                            nc.tensor.matmul(
                                vt_e[:, :], x3d[0:C3N, we, :],
                                wqv3d[0:C3N, WVOFF:WVOFF + C],
                                start=False, stop=True,
                            )
                            nc.tensor.matmul(
                                vt_o[:, :], x3d[64:64 + C3N, wo, :],
                                wqv3d[64:64 + C3N, WVOFF:WVOFF + C],
                                start=False, stop=True,
                            )
                            # column scores + exp + mask
                            for wi, vt_ps in ((we, vt_e), (wo, vt_o)):
                                fo = (w0 + wi) * H
                                st_ps = stp.tile([128, 128], f32, tag="st")
                                nc.tensor.matmul(
                                    st_ps[:, :], k_sb[:, fo:fo + H],
                                    q_sb[:, fo:fo + H], start=True, stop=True,
                                )
                                est = p1s.tile([128, 128], bf16, tag="est")
                                nc.scalar.activation(
                                    est[:, :], st_ps[:, :], AF.Exp
                                )
                                nc.gpsimd.tensor_mul(
                                    out=est[:, :], in0=est[:, :],
                                    in1=mask_sb[:, :],
                                )
                                ests[wi] = est
                                nc.vector.tensor_copy(
                                    out=vtg[:, wi, 0:C], in_=vt_ps[:, :]
                                )
                        if pp >= 1:
                            for wj in (2 * (pp - 1), 2 * (pp - 1) + 1):
                                uh_ps = uhp.tile([128, C + 1], f32, tag="uh")
                                nc.tensor.matmul(
                                    uh_ps[:, :], ests[wj][:, :], vtg[:, wj, :],
                                    start=True, stop=True,
                                )
                                nc.scalar.copy(
                                    out=uhg[:, wj, 0:144], in_=uh_ps[:, 0:144]
                                )
                                nc.vector.tensor_copy(
                                    out=uhg[:, wj, 144:C + 1],
                                    in_=uh_ps[:, 144:C + 1],
                                )

                    nc.gpsimd.dma_start(
                        out=vt_st[:, w0:w0 + gw, :], in_=vtg[:, 0:gw, :]
                    )
                    nc.gpsimd.dma_start(
                        out=uh_st[:, w0:w0 + gw, :], in_=uhg[:, 0:gw, :]
                    )
                    w0 += gw

            # ---------------- Phase 2: row branch + combine (per h) ---------
            with tc.tile_pool(name="p2b", bufs=2) as p2b, \
                 tc.tile_pool(name="p2s", bufs=4) as p2s, \
                 tc.tile_pool(name="p2r", bufs=8) as p2r, \
                 tc.tile_pool(name="stp2", bufs=3, space="PSUM") as stp2, \
                 tc.tile_pool(name="uwp", bufs=5, space="PSUM") as uwp:
                h0 = 0
                for gh in P2_GROUPS:
                    vtr = p2b.tile([W, GH, C + 1], bf16, tag="vtr")
                    nc.sync.dma_start(
                        out=vtr[:, 0:gh, :],
                        in_=vt_st[h0:h0 + gh, :, :].rearrange("h w c -> w h c"),
                    )
                    uhr = p2b.tile([W, GH, C + 1], bf16, tag="uhr")
                    nc.scalar.dma_start(
                        out=uhr[:, 0:gh, :],
                        in_=uh_st[h0:h0 + gh, :, :].rearrange("h w c -> w h c"),
                    )
                    xtr = p2b.tile([W, GH, C], bf16, tag="xtr")
                    nc.sync.dma_start(
                        out=xtr[:, 0:gh, :], in_=xt[:, h0:h0 + gh, :]
                    )
                    og = p2b.tile([W, GH, C], bf16, tag="og")

                    estws = [None] * GH
                    for hi in range(gh + 2):
                        if hi < gh:
                            h = h0 + hi
                            stw_ps = stp2.tile([128, 128], f32, tag="stw")
                            nc.tensor.matmul(
                                stw_ps[:, :], k_v[:, h, :], q_v[:, h, :],
                                start=True, stop=True,
                            )
                            estw = p2s.tile([128, 128], bf16, tag="estw")
                            nc.scalar.activation(estw[:, :], stw_ps[:, :], AF.Exp)
                            estws[hi] = estw
                        if hi >= 2:
                            hj = hi - 2
                            # UW|ZW' then accumulate loaded UH|ZH' via identity
                            uw_ps = uwp.tile([128, C + 1], f32, tag="uw")
                            nc.tensor.matmul(
                                uw_ps[:, :], estws[hj][:, :], vtr[:, hj, :],
                                start=True, stop=False,
                            )
                            nc.tensor.matmul(
                                uw_ps[:, :], identb[:, :], uhr[:, hj, :],
                                start=False, stop=True,
                            )
                            rz = p2r.tile([128, 1], f32, tag="rz")
                            nc.vector.reciprocal(
                                out=rz[:, :], in_=uw_ps[:, C:C + 1]
                            )
                            # og = (UH+UW) * (gamma/Z) + fac*x in one DVE op
                            nc.vector.scalar_tensor_tensor(
                                out=og[:, hj, :], in0=uw_ps[:, 0:C],
                                scalar=rz[:, :], in1=xtr[:, hj, :],
                                op0=mybir.AluOpType.mult,
                                op1=mybir.AluOpType.add,
                            )

                    nc.sync.dma_start(
                        out=out[:, h0:h0 + gh, :], in_=og[:, 0:gh, :]
                    )
                    h0 += gh

    nc.compile()
    return nc


def _get_nc():
    if "nc" not in _CACHE:
        _CACHE["nc"] = _build_nc()
    return _CACHE["nc"]


def _prep_inputs(x, Wq, bq, Wk, bk, Wv, bv, eca_w, gamma):
    x = np.asarray(x, np.float32)
    eca_w = np.asarray(eca_w, np.float32)
    gamma = float(np.asarray(gamma, np.float32).reshape(()))

    wqv = np.zeros((353, 388), np.float32)
    wqv[0:C, 0:CQ] = np.asarray(Wq, np.float32).T
    wqv[C, 0:CQ] = np.asarray(bq, np.float32)
    wqv[0:C, KOFF:KOFF + CQ] = np.asarray(Wk, np.float32).T
    wqv[C, KOFF:KOFF + CQ] = np.asarray(bk, np.float32)
    wqv[0:C, WVOFF:WVOFF + C] = np.asarray(Wv, np.float32).T
    wqv[C, WVOFF:WVOFF + C] = np.asarray(bv, np.float32)
    wqv[320:353, :] = wqv[256:289, :]
    wqv = wqv.astype(BF16)

    rg = np.asarray(1.0 / gamma, np.float32).reshape(1, 1)

    # host-side ECA channel factor: fac = 1 + gamma * sigmoid(conv1d(mean(x)))
    y = x.mean(axis=(2, 3))                      # [b, c]
    yp = np.pad(y, ((0, 0), (1, 1)))
    yc = (eca_w[0] * yp[:, :-2] + eca_w[1] * yp[:, 1:-1]
          + eca_w[2] * yp[:, 2:])
    facv = (1.0 + gamma / (1.0 + np.exp(-yc))).astype(np.float32)  # [b, c]

    ones_plane = np.ones((1, W, H), np.float32)
    zeros_pad = np.zeros((31, W, H), np.float32)
    in_maps = []
    for b in range(B):
        xb = x[b]                                           # [c, h, w]
        xcw = xb.transpose(0, 2, 1)                         # [c, w, h]
        xwv = np.concatenate(
            [xcw, ones_plane, zeros_pad, xcw[256:288], ones_plane]
        ).astype(BF16)
        # xt ships with the ECA channel factor pre-applied
        xtf = xb * facv[b][:, None, None]
        xtv = np.ascontiguousarray(xtf.transpose(2, 1, 0)).astype(BF16)
        in_maps.append({
            "xw": xwv, "xt": xtv, "wqv": wqv, "rg": rg,
        })
    return in_maps


def kernel(x, Wq, bq, Wk, bk, Wv, bv, eca_w, gamma, _return_results=False,
           **run_kwargs):
    from concourse.bass_utils import run_bass_kernel_spmd

    nc = _get_nc()
    in_maps = _prep_inputs(x, Wq, bq, Wk, bk, Wv, bv, eca_w, gamma)
    res = run_bass_kernel_spmd(nc, in_maps, core_ids=list(range(B)), **run_kwargs)
    out = np.empty((B, C, H, W), np.float32)
    for b in range(B):
        # device output is [w, h, c]
        out[b] = res.results[b]["out"].astype(np.float32).transpose(2, 1, 0)
    if _return_results:
        return out, res
    return out

